# revision 5
# baseline (speedup 1.0000x reference)
"""Trainium2 Bass kernel for GQA causal self-attention (RMS-norm QK + NTK RoPE + proj).

Sharding: 8 cores = 2 batches x 4 KV-head groups. Each core computes QKV
projections (bf16 matmuls, f32 accumulate), RMS-norm + NTK RoPE + gain, and
causal attention in a transposed-softmax formulation (scores are bounded since
q/k are RMS-normalized, so no max subtraction; softmax row sums come from a
ones-column matmul) for its (batch, kv-group). A single 8-way AllToAll then
redistributes attention outputs so each core holds full features for a disjoint
T/8-row slice of BOTH batches, and computes that slice of the output projection
(Wproj in bf16). Host only transposes/slices/concats - all FLOPs are on device.
"""

import sys

for _p in ("/opt/trn_rl_repo", "/root/.axon_site/_ro/trn_rl_repo"):
    if _p not in sys.path:
        sys.path.append(_p)

import numpy as np
import ml_dtypes

import concourse.bass as bass
import concourse.mybir as mybir
import concourse.tile as tile
from concourse import bacc
from concourse.bass import ts, ds
from concourse.bass_utils import run_bass_kernel_spmd

FP32 = mybir.dt.float32
BF16 = mybir.dt.bfloat16
AF = mybir.ActivationFunctionType
OP = mybir.AluOpType

B, DIM, H, HKV = 2, 2048, 16, 4
D = 128
HALF = D // 2
G = H // HKV  # q heads per kv head (= heads per core)
HL = G  # 4 local q heads
ROPE_BASE = 10000.0
TRAIN_LEN = 1024
EPS = float(np.finfo(np.float32).eps)
NCORES = 8
GRP = 4  # cores per batch group


def build_nc(
    T: int, use_collective: bool = True, phases: int = 4, p1stop: int = 60
) -> bass.Bass:
    """Build the per-core Bass program (same program on all 8 cores)."""
    TB = T // 128  # t-blocks
    CH = min(512, T)  # tq chunk width for attention
    NCH = T // CH
    SUB = CH // 128  # 128-blocks per chunk
    TSL = T // NCORES  # sequence slice (per batch) per core after AllToAll
    MT = min(128, TSL)  # proj output t-block partition size
    NTB4 = TSL // MT
    OCH = 512  # proj output column chunk
    NOCH = DIM // OCH
    FO = DIM // 128  # feature k-tiles
    ISQ = 1.0 / float(np.sqrt(D))

    nc = bacc.Bacc("TRN2", target_bir_lowering=False, debug=False, num_devices=NCORES)

    xT = nc.dram_tensor("xT", [DIM, T], BF16, kind="ExternalInput")
    wq = nc.dram_tensor("wq", [DIM, HL * D], BF16, kind="ExternalInput")
    wkv = nc.dram_tensor("wkv", [DIM, 2 * D], BF16, kind="ExternalInput")
    wp = nc.dram_tensor("wp", [DIM, DIM], BF16, kind="ExternalInput")
    cost = nc.dram_tensor("cost", [T, HALF], FP32, kind="ExternalInput")
    sint = nc.dram_tensor("sint", [T, HALF], FP32, kind="ExternalInput")
    gain = nc.dram_tensor("gain", [128, HL], FP32, kind="ExternalInput")
    trimask = nc.dram_tensor("trimask", [128, 128], BF16, kind="ExternalInput")
    ident = nc.dram_tensor("ident", [128, 128], FP32, kind="ExternalInput")
    out = nc.dram_tensor("out", [B, TSL, DIM], FP32, kind="ExternalOutput")

    rg = [list(range(NCORES))]

    with tile.TileContext(nc) as tc:
        with (
            tc.tile_pool(name="consts", bufs=1) as consts,
            tc.tile_pool(name="persist", bufs=1) as persist,
            tc.tile_pool(name="dram", bufs=1, space="DRAM") as dram,
        ):
            # ---- resident constants / weights ----
            wq_sb = consts.tile([128, FO, HL * D], BF16)
            nc.sync.dma_start(wq_sb[:], wq.ap().rearrange("(fo fi) h -> fi fo h", fi=128))
            wkv_sb = consts.tile([128, FO, 2 * D], BF16)
            nc.sync.dma_start(wkv_sb[:], wkv.ap().rearrange("(fo fi) h -> fi fo h", fi=128))
            cos_sb = consts.tile([128, TB, HALF], FP32)
            nc.sync.dma_start(cos_sb[:], cost.ap().rearrange("(tb p) h -> p tb h", p=128))
            sin_sb = consts.tile([128, TB, HALF], FP32)
            nc.sync.dma_start(sin_sb[:], sint.ap().rearrange("(tb p) h -> p tb h", p=128))
            gain_sb = consts.tile([128, HL], FP32)
            nc.sync.dma_start(gain_sb[:], gain.ap())
            tri_sb = consts.tile([128, 128], BF16)
            nc.sync.dma_start(tri_sb[:], trimask.ap())
            id_sb = consts.tile([128, 128], FP32)
            nc.sync.dma_start(id_sb[:], ident.ap())
            ones_sb = consts.tile([128, 1], BF16)
            nc.vector.memset(ones_sb[:], 1.0)
            eps_sb = consts.tile([128, 1], FP32)
            nc.vector.memset(eps_sb[:], EPS)

            # ---- persistent activation buffers ----
            qT_sb = persist.tile([128, HL, T], BF16)  # q, d-major per head
            kT_sb = persist.tile([128, T], BF16)  # k, d-major
            v_sb = persist.tile([128, TB, D], BF16)  # v, t-major tiles

            a2a_in = dram.tile([NCORES, HL, D, TSL], BF16, name="a2a_in")
            a2a_out = dram.tile([NCORES, HL, D, TSL], BF16, name="a2a_out")

            # =============== Phase 1: QKV + norm + rope + transpose ===============
            with (
                tc.tile_pool(name="p1sb", bufs=2) as p1sb,
                tc.tile_pool(name="p1ps", bufs=2, space="PSUM") as p1ps,
                tc.tile_pool(name="p1tp", bufs=2, space="PSUM") as p1tp,
            ):
                kt_ps = None
                for tb in range(TB):
                    xt = p1sb.tile([128, FO, 128], BF16, tag="xt")
                    nc.sync.dma_start(
                        xt[:], xT.ap().rearrange("(fo fi) t -> fi fo t", fi=128)[:, :, ts(tb, 128)]
                    )
                    q_ps = p1ps.tile([128, HL * D], FP32, tag="q_ps")
                    kv_ps = p1ps.tile([128, 2 * D], FP32, tag="kv_ps")
                    for fo in range(FO):
                        nc.tensor.matmul(
                            q_ps[:], xt[:, fo, :], wq_sb[:, fo, :],
                            start=(fo == 0), stop=(fo == FO - 1),
                        )
                    for fo in range(FO):
                        nc.tensor.matmul(
                            kv_ps[:], xt[:, fo, :], wkv_sb[:, fo, :],
                            start=(fo == 0), stop=(fo == FO - 1),
                        )

                    # stage to SBUF
                    q_sb = p1sb.tile([128, HL * D], FP32, tag="q_sb")
                    nc.scalar.copy(q_sb[:], q_ps[:])
                    k_sb = p1sb.tile([128, D], FP32, tag="k_sb")
                    nc.vector.tensor_copy(k_sb[:], kv_ps[:, :D])
                    nc.vector.tensor_copy(v_sb[:, tb, :], kv_ps[:, D:])

                    if p1stop < 20:
                        continue
                    # rms stats: rms = sqrt(sumsq/D + eps) per head (q: 0..3, k: 4)
                    sumsq = p1sb.tile([128, HL + 1], FP32, tag="sumsq")
                    scrq = p1sb.tile([128, HL * D], FP32, tag="scrq")
                    nc.scalar.square(scrq[:], q_sb[:])
                    scrk = p1sb.tile([128, D], FP32, tag="scrk")
                    nc.scalar.square(scrk[:], k_sb[:])
                    nc.vector.tensor_reduce(
                        sumsq[:, :HL],
                        scrq[:].rearrange("p (h d) -> p h d", d=D),
                        mybir.AxisListType.X, OP.add,
                    )
                    nc.vector.tensor_reduce(
                        sumsq[:, HL : HL + 1], scrk[:], mybir.AxisListType.X, OP.add
                    )
                    rms = p1sb.tile([128, HL + 1], FP32, tag="rms")
                    if p1stop < 21:
                        continue
                    nc.scalar.activation(
                        rms[:], sumsq[:], AF.Sqrt, bias=eps_sb[:], scale=1.0 / D
                    )
                    scl = p1sb.tile([128, HL + 1], FP32, tag="scl")
                    if p1stop < 22:
                        continue
                    nc.vector.reciprocal(scl[:], rms[:])
                    sclg = p1sb.tile([128, HL], FP32, tag="sclg")
                    if p1stop < 23:
                        continue
                    nc.vector.tensor_mul(sclg[:], scl[:, :HL], gain_sb[:])

                    if p1stop < 30:
                        continue
                    # rope on raw q/k (norm scale applied after; it commutes)
                    q4 = q_sb[:].rearrange("p (h two half) -> p h two half", two=2, half=HALF)
                    cos_b = cos_sb[:, tb, None, None, :].to_broadcast([128, HL, 2, HALF])
                    sin_b = sin_sb[:, tb, None, None, :].to_broadcast([128, HL, 2, HALF])
                    qa = p1sb.tile([128, HL, 2, HALF], FP32, tag="qa")
                    qb = p1sb.tile([128, HL, 2, HALF], FP32, tag="qb")
                    nc.vector.tensor_mul(qa[:], q4, cos_b)
                    nc.vector.tensor_mul(qb[:], q4, sin_b)
                    q_rot = p1sb.tile([128, HL, 2, HALF], FP32, tag="q_rot")
                    nc.vector.tensor_add(q_rot[:, :, 0, :], qa[:, :, 0, :], qb[:, :, 1, :])
                    nc.vector.tensor_sub(q_rot[:, :, 1, :], qa[:, :, 1, :], qb[:, :, 0, :])

                    if p1stop < 40:
                        continue
                    k2 = k_sb[:].rearrange("p (two half) -> p two half", two=2)
                    cos_k = cos_sb[:, tb, None, :].to_broadcast([128, 2, HALF])
                    sin_k = sin_sb[:, tb, None, :].to_broadcast([128, 2, HALF])
                    ka = p1sb.tile([128, 2, HALF], FP32, tag="ka")
                    kb = p1sb.tile([128, 2, HALF], FP32, tag="kb")
                    nc.gpsimd.tensor_mul(ka[:], k2, cos_k)
                    nc.gpsimd.tensor_mul(kb[:], k2, sin_k)
                    k_rot = p1sb.tile([128, 2, HALF], FP32, tag="k_rot")
                    nc.gpsimd.tensor_add(k_rot[:, 0, :], ka[:, 0, :], kb[:, 1, :])
                    nc.gpsimd.tensor_sub(k_rot[:, 1, :], ka[:, 1, :], kb[:, 0, :])

                    if p1stop < 50:
                        continue
                    # apply rms scale (and gain for q)
                    q_fin = p1sb.tile([128, HL * D], FP32, tag="q_fin")
                    qr2 = q_rot[:].rearrange("p h two half -> p (h two half)")
                    for h in range(HL):
                        nc.scalar.activation(
                            q_fin[:, ts(h, D)], qr2[:, ts(h, D)], AF.Copy,
                            scale=sclg[:, h : h + 1],
                        )
                    k_fin = p1sb.tile([128, D], FP32, tag="k_fin")
                    nc.gpsimd.tensor_scalar_mul(
                        k_fin[:], k_rot[:].rearrange("p two half -> p (two half)"),
                        scl[:, HL : HL + 1],
                    )

                    if p1stop < 60:
                        continue
                    # transpose to d-major
                    qt_ps = p1tp.tile([128, HL * D], FP32, tag="qt_ps")
                    for h in range(HL):
                        nc.tensor.transpose(qt_ps[:, ts(h, D)], q_fin[:, ts(h, D)], id_sb[:])
                    nc.vector.tensor_copy(
                        qT_sb[:, :, ts(tb, 128)],
                        qt_ps[:].rearrange("p (h t) -> p h t", h=HL),
                    )
                    if tb % 4 == 0:
                        kt_ps = p1tp.tile([128, 4 * D], FP32, tag="kt_ps")
                    nc.tensor.transpose(kt_ps[:, ts(tb % 4, D)], k_fin[:], id_sb[:])
                    if tb % 4 == 3 or tb == TB - 1:
                        nb = tb % 4 + 1
                        nc.vector.tensor_copy(
                            kT_sb[:, ds((tb - nb + 1) * 128, nb * 128)], kt_ps[:, : nb * 128]
                        )

            if phases <= 1:
                # debug: dump v (and qT when built) so phase 1 stays live
                nelem = B * TSL * DIM
                with tc.tile_pool(name="dbg", bufs=1) as dbg:
                    dt = dbg.tile([128, nelem // 128], FP32)
                    nc.vector.memset(dt[:], 0.0)
                    nc.vector.tensor_copy(
                        dt[:, : TB * D], v_sb[:].rearrange("p tb d -> p (tb d)")
                    )
                    if p1stop >= 60:
                        nc.vector.tensor_copy(
                            dt[:], qT_sb[:].rearrange("p h t -> p (h t)")[:, : nelem // 128]
                        )
                    nc.sync.dma_start(
                        out.ap()
                        .rearrange("b t o -> (b t o)")
                        .rearrange("(p f) -> p f", p=128),
                        dt[:],
                    )

            # =============== Phase 2: causal attention (transposed softmax) ========
            with (
                tc.tile_pool(name="p2sb", bufs=3) as p2sb,
                tc.tile_pool(name="p2sp", bufs=2, space="PSUM") as p2sp,
                tc.tile_pool(name="p2op", bufs=2, space="PSUM") as p2op,
                tc.tile_pool(name="p2mp", bufs=2, space="PSUM") as p2mp,
            ):
                for h in range(HL if phases >= 2 else 0):
                    for c in range(NCH):
                        nblk = SUB * c + SUB  # total tk blocks for this chunk
                        o_ps = p2op.tile([128, CH], FP32, tag="o_ps")
                        sum_ps = p2mp.tile([1, CH], FP32, tag="sum_ps")
                        for j in range(nblk):
                            dj = j - SUB * c  # >= 0 on diagonal blocks
                            pT = p2sb.tile([128, CH], BF16, tag="pT")
                            s_ps = p2sp.tile([128, CH], FP32, tag="s_ps")
                            if dj < 0:
                                nc.tensor.matmul(
                                    s_ps[:], kT_sb[:, ts(j, 128)],
                                    qT_sb[:, h, ds(c * CH, CH)],
                                    start=True, stop=True,
                                )
                                nc.scalar.activation(pT[:], s_ps[:], AF.Exp, scale=ISQ)
                            else:
                                off = dj * 128
                                w = CH - off
                                nc.tensor.matmul(
                                    s_ps[:, off:CH], kT_sb[:, ts(j, 128)],
                                    qT_sb[:, h, ds(c * CH + off, w)],
                                    start=True, stop=True,
                                )
                                nc.scalar.activation(
                                    pT[:, off:CH], s_ps[:, off:CH], AF.Exp, scale=ISQ
                                )
                                if off > 0:
                                    nc.vector.memset(pT[:, :off], 0.0)
                                nc.vector.tensor_mul(
                                    pT[:, off : off + 128], pT[:, off : off + 128], tri_sb[:]
                                )
                            nc.tensor.matmul(
                                o_ps[:], v_sb[:, j, :], pT[:],
                                start=(j == 0), stop=(j == nblk - 1),
                            )
                            nc.tensor.matmul(
                                sum_ps[:], ones_sb[:], pT[:],
                                start=(j == 0), stop=(j == nblk - 1),
                            )
                        rs = p2sb.tile([1, CH], FP32, tag="rs")
                        nc.vector.reciprocal(rs[:], sum_ps[:])
                        rb = p2sb.tile([128, CH], FP32, tag="rb")
                        nc.gpsimd.partition_broadcast(rb[:], rs[:])
                        stage = p2sb.tile([128, CH], BF16, tag="stage")
                        nc.vector.tensor_mul(stage[:], o_ps[:], rb[:])
                        nsl = CH // TSL
                        for sl in range(nsl):
                            nc.sync.dma_start(
                                a2a_in[c * nsl + sl, h, :, :],
                                stage[:, ts(sl, TSL)],
                            )

            # =============== Phase 3: AllToAll across all 8 cores =============
            if phases < 3:
                pass
            elif use_collective:
                nc.gpsimd.collective_compute(
                    "AllToAll", OP.bypass, replica_groups=rg,
                    ins=[a2a_in[:]], outs=[a2a_out[:]],
                )
            else:
                nc.sync.dma_start(a2a_out[:], a2a_in[:])

            # =============== Phase 4: output projection (row-sharded) =============
            with (
                tc.tile_pool(name="p4sb", bufs=2) as p4sb,
                tc.tile_pool(name="p4in", bufs=1) as p4in,
                tc.tile_pool(name="p4ps", bufs=4, space="PSUM") as p4ps,
            ):
                pin_sb = p4in.tile([128, B, FO, TSL], BF16)
                for beta in range(B if phases >= 4 else 0):
                    for g in range(GRP):
                        for h in range(HL):
                            nc.sync.dma_start(
                                pin_sb[:, beta, g * HL + h],
                                a2a_out[beta * GRP + g, h],
                            )
                for oc in range(NOCH if phases >= 4 else 0):
                    wp_sb = p4sb.tile([128, FO, OCH], BF16, tag="wp_sb")
                    nc.sync.dma_start(
                        wp_sb[:],
                        wp.ap().rearrange("(fo fi) o -> fi fo o", fi=128)[:, :, ts(oc, OCH)],
                    )
                    for beta in range(B):
                        for tb in range(NTB4):
                            pr_ps = p4ps.tile([MT, OCH], FP32, tag="pr_ps")
                            for fo in range(FO):
                                nc.tensor.matmul(
                                    pr_ps[:], pin_sb[:, beta, fo, ts(tb, MT)],
                                    wp_sb[:, fo, :],
                                    start=(fo == 0), stop=(fo == FO - 1),
                                )
                            o_sb = p4sb.tile([MT, OCH], FP32, tag="o_sb")
                            if tb % 2 == 0:
                                nc.vector.tensor_copy(o_sb[:], pr_ps[:])
                            else:
                                nc.scalar.copy(o_sb[:], pr_ps[:])
                            nc.sync.dma_start(
                                out.ap()[beta, ts(tb, MT), ts(oc, OCH)], o_sb[:]
                            )

    nc.compile()
    return nc


def _rope_tables(T: int):
    if T > TRAIN_LEN:
        scale = T / TRAIN_LEN
        base = ROPE_BASE * scale ** (D / (D - 2))
    else:
        base = ROPE_BASE
    inv_freq = 1.0 / base ** (np.arange(0, D, 2, dtype=np.float32) / D)
    freqs = np.outer(np.arange(T, dtype=np.float32), inv_freq)
    return (
        np.cos(freqs).astype(np.float32),
        np.sin(freqs).astype(np.float32),
    )


def make_in_maps(x, Wq, Wk, Wv, Wproj, q_gain, T: int):
    cos, sin = _rope_tables(T)
    tri = np.triu(np.ones((128, 128), dtype=ml_dtypes.bfloat16))
    ident = np.eye(128, dtype=np.float32)
    wpT = np.ascontiguousarray(Wproj.T).astype(ml_dtypes.bfloat16)
    xTs = [np.ascontiguousarray(x[b].T).astype(ml_dtypes.bfloat16) for b in range(x.shape[0])]
    in_maps = []
    for c in range(NCORES):
        b, g = c // GRP, c % GRP
        wq_c = np.ascontiguousarray(Wq[g * HL * D : (g + 1) * HL * D, :].T).astype(
            ml_dtypes.bfloat16
        )
        wkv_c = np.ascontiguousarray(
            np.concatenate([Wk[g * D : (g + 1) * D, :], Wv[g * D : (g + 1) * D, :]], axis=0).T
        ).astype(ml_dtypes.bfloat16)
        gain_c = np.broadcast_to(
            q_gain[g * HL : (g + 1) * HL][None, :], (128, HL)
        ).astype(np.float32).copy()
        in_maps.append(
            {
                "xT": xTs[b],
                "wq": wq_c,
                "wkv": wkv_c,
                "wp": wpT,
                "cost": cos,
                "sint": sin,
                "gain": gain_c,
                "trimask": tri,
                "ident": ident,
            }
        )
    return in_maps


_NC_CACHE = {}


def run(x, Wq, Wk, Wv, Wproj, q_gain, T=None, use_collective=True, **spmd_kwargs):
    T = T if T is not None else x.shape[1]
    key = (T, use_collective)
    if key not in _NC_CACHE:
        _NC_CACHE[key] = build_nc(T, use_collective)
    nc = _NC_CACHE[key]
    in_maps = make_in_maps(x, Wq, Wk, Wv, Wproj, q_gain, T)
    res = run_bass_kernel_spmd(nc, in_maps, core_ids=list(range(NCORES)), **spmd_kwargs)
    TSL = T // NCORES
    out = np.empty((x.shape[0], T, DIM), dtype=np.float32)
    for c in range(NCORES):
        out[:, c * TSL : (c + 1) * TSL, :] = res.results[c]["out"]
    return out, res


def kernel(x, Wq, Wk, Wv, Wproj, q_gain):
    x = np.asarray(x, dtype=np.float32)
    out, _ = run(
        x,
        np.asarray(Wq, dtype=np.float32),
        np.asarray(Wk, dtype=np.float32),
        np.asarray(Wv, dtype=np.float32),
        np.asarray(Wproj, dtype=np.float32),
        np.asarray(q_gain, dtype=np.float32),
    )
    return out



# revision 6
# speedup vs baseline: 1.4928x; 1.4928x over previous
"""Trainium2 Bass kernel for GQA causal self-attention — zero-collective design.

Sharding: 8 cores = 2 batches x 4 interleaved query-row sets. Core c handles
batch b=c//4 and query rows t = 512*i + s + 4*r (s=c%4, i in 0..3, r in 0..127),
a causally balanced quarter of the rows. Each core computes, end to end with no
cross-core communication: Q projection for its rows (all 16 heads, feat-major),
full K/V projection (all 4 kv heads, duplicated across the 4 cores of a batch),
RMS-norm + NTK RoPE (feat-major, via transposed cos/sin tables; K's norm is
folded into the per-key exp scale), causal attention in transposed-softmax form
(row sums via a ones-column matmul), and the output projection for its rows.
The causal structure is SPMD-uniform: key block j covers chunk columns >=
128*(j//4) + 32*(j%4), with a per-core data mask handling the ragged boundary.
"""

import sys

for _p in ("/opt/trn_rl_repo", "/root/.axon_site/_ro/trn_rl_repo"):
    if _p not in sys.path:
        sys.path.append(_p)

import numpy as np
import ml_dtypes

import concourse.bass as bass
import concourse.mybir as mybir
import concourse.tile as tile
from concourse import bacc
from concourse.bass import ts, ds
from concourse.bass_utils import run_bass_kernel_spmd

FP32 = mybir.dt.float32
BF16 = mybir.dt.bfloat16
AF = mybir.ActivationFunctionType
OP = mybir.AluOpType

B, DIM, H, HKV = 2, 2048, 16, 4
D = 128
HALF = D // 2
G = H // HKV
ROPE_BASE = 10000.0
TRAIN_LEN = 1024
EPS = float(np.finfo(np.float32).eps)
NCORES = 8
GRP = 4  # cores per batch


def build_nc(T: int) -> bass.Bass:
    TSL = T // GRP  # 512 query rows per core
    NI = TSL // 128  # 4 gathered q blocks
    TB = T // 128  # 16 key blocks
    FO = DIM // 128  # 16 contraction chunks
    FKV = (HKV * D * 2) // 128  # 8 kv feature blocks (4 K heads + 4 V heads)
    NTCH = T // 512  # 4 key-t chunks for KV proj
    NOC = DIM // 512  # 4 output column chunks
    DEPS = float(D) * EPS

    nc = bacc.Bacc("TRN2", target_bir_lowering=False, debug=False, num_devices=NCORES)

    xT = nc.dram_tensor("xT", [DIM, T], BF16, kind="ExternalInput")
    xTg = nc.dram_tensor("xTg", [DIM, TSL], BF16, kind="ExternalInput")
    wq = nc.dram_tensor("wq", [DIM, DIM], BF16, kind="ExternalInput")
    wkv = nc.dram_tensor("wkv", [DIM, HKV * D * 2], BF16, kind="ExternalInput")
    wp = nc.dram_tensor("wp", [DIM, DIM], BF16, kind="ExternalInput")
    cosq = nc.dram_tensor("cosq", [128, TSL], FP32, kind="ExternalInput")
    sinq = nc.dram_tensor("sinq", [128, TSL], FP32, kind="ExternalInput")
    cosk = nc.dram_tensor("cosk", [128, T], FP32, kind="ExternalInput")
    sink = nc.dram_tensor("sink", [128, T], FP32, kind="ExternalInput")
    maskd = nc.dram_tensor("maskd", [128, 512], BF16, kind="ExternalInput")
    oh4 = nc.dram_tensor("oh4", [128, 16], BF16, kind="ExternalInput")
    idf = nc.dram_tensor("idf", [128, 128], FP32, kind="ExternalInput")
    idb = nc.dram_tensor("idb", [128, 128], BF16, kind="ExternalInput")
    gsd = nc.dram_tensor("gsd", [1, H], FP32, kind="ExternalInput")
    out = nc.dram_tensor("out", [TSL, DIM], FP32, kind="ExternalOutput")

    with tile.TileContext(nc) as tc:
        with (
            tc.tile_pool(name="consts", bufs=1) as consts,
            tc.tile_pool(name="persist", bufs=1) as persist,
        ):
            cosq_sb = consts.tile([128, TSL], FP32)
            nc.sync.dma_start(cosq_sb[:], cosq.ap())
            sinq_sb = consts.tile([128, TSL], FP32)
            nc.sync.dma_start(sinq_sb[:], sinq.ap())
            cosk_sb = consts.tile([128, T], FP32)
            nc.sync.dma_start(cosk_sb[:], cosk.ap())
            sink_sb = consts.tile([128, T], FP32)
            nc.sync.dma_start(sink_sb[:], sink.ap())
            mask_sb = consts.tile([128, 512], BF16)
            nc.sync.dma_start(mask_sb[:], maskd.ap())
            oh4_sb = consts.tile([128, 16], BF16)
            nc.sync.dma_start(oh4_sb[:], oh4.ap())
            idf_sb = consts.tile([128, 128], FP32)
            nc.sync.dma_start(idf_sb[:], idf.ap())
            idb_sb = consts.tile([128, 128], BF16)
            nc.sync.dma_start(idb_sb[:], idb.ap())
            gsd_sb = consts.tile([1, H], FP32)
            nc.sync.dma_start(gsd_sb[:], gsd.ap())
            ones_sb = consts.tile([128, 1], BF16)
            nc.vector.memset(ones_sb[:], 1.0)
            epsd_sb = consts.tile([128, 1], FP32)
            nc.vector.memset(epsd_sb[:], DEPS)

            xg_sb = persist.tile([128, FO, TSL], BF16)
            nc.sync.dma_start(xg_sb[:], xTg.ap().rearrange("(fo fi) t -> fi fo t", fi=128))

            kT_sb = persist.tile([128, HKV, T], BF16)  # K, feat-major, rope'd
            v_sb = persist.tile([128, HKV, TB, D], BF16)  # V, t-major blocks
            qT_sb = persist.tile([128, H, TSL], BF16)  # Q-hat, feat-major
            yT_sb = persist.tile([128, H, TSL], BF16)  # attn out, feat-major
            bT_sb = persist.tile([128, TB, HKV], FP32)  # per-key exp scale
            binv_sb = persist.tile([HKV, T], FP32)

            # =============== Phase Q: q projection + rms + rope + gain ===========
            with (
                tc.tile_pool(name="qw", bufs=2) as qw,
                tc.tile_pool(name="qs", bufs=3) as qs,
                tc.tile_pool(name="qp", bufs=2, space="PSUM") as qp,
                tc.tile_pool(name="qq", bufs=2, space="PSUM") as qq,
            ):
                pend_q = None
                for h in range(H):
                    wh = qw.tile([128, FO, 128], BF16, tag="wh")
                    nc.sync.dma_start(
                        wh[:], wq.ap().rearrange("(fo fi) o -> fi fo o", fi=128)[:, :, ts(h, 128)]
                    )
                    ps = qp.tile([128, TSL], FP32, tag="qps")
                    for fo in range(FO):
                        nc.tensor.matmul(
                            ps[:], wh[:, fo, :], xg_sb[:, fo, :],
                            start=(fo == 0), stop=(fo == FO - 1),
                        )
                    if pend_q is not None:
                        pend_q()
                        pend_q = None
                    sq = qs.tile([128, TSL], BF16, tag="qsq")
                    nc.scalar.square(sq[:], ps[:])
                    # rope on raw q (norm/gain applied after; rotation commutes)
                    ca = qs.tile([128, TSL], FP32, tag="qca")
                    nc.vector.tensor_mul(ca[:], ps[:], cosq_sb[:])
                    cb = qs.tile([128, TSL], FP32, tag="qcb")
                    nc.vector.tensor_mul(cb[0:HALF, :], ps[HALF:128, :], sinq_sb[0:HALF, :])
                    nc.vector.tensor_mul(cb[HALF:128, :], ps[0:HALF, :], sinq_sb[HALF:128, :])
                    rot = qs.tile([128, TSL], FP32, tag="rot")
                    nc.vector.tensor_add(rot[:], ca[:], cb[:])

                    def _fin(sq=sq, rot=rot, h=h):
                        qss = qq.tile([1, TSL], FP32, tag="qss")
                        nc.tensor.matmul(qss[:], ones_sb[:], sq[:], start=True, stop=True)
                        srt = qs.tile([1, TSL], FP32, tag="qsrt")
                        nc.scalar.activation(srt[:], qss[:], AF.Sqrt, bias=epsd_sb[0:1, :])
                        rcp = qs.tile([1, TSL], FP32, tag="qrcp")
                        nc.vector.reciprocal(rcp[:], srt[:])
                        alf = qs.tile([1, TSL], FP32, tag="alf")
                        nc.vector.tensor_scalar_mul(alf[:], rcp[:], gsd_sb[0:1, h : h + 1])
                        ab = qs.tile([128, TSL], FP32, tag="ab")
                        nc.gpsimd.partition_broadcast(ab[:], alf[:])
                        nc.vector.tensor_mul(qT_sb[:, h, :], rot[:], ab[:])

                    pend_q = _fin
                pend_q()

            # =============== Phase KV: full k/v projection + rope + norm scale ====
            with (
                tc.tile_pool(name="kvw", bufs=2) as kvw,
                tc.tile_pool(name="kvx", bufs=2) as kvx,
                tc.tile_pool(name="kvs", bufs=3) as kvs,
                tc.tile_pool(name="kvp", bufs=2, space="PSUM") as kvp,
                tc.tile_pool(name="kvq", bufs=2, space="PSUM") as kvq,
            ):
                for tch in range(NTCH):
                    xt = kvx.tile([128, FO, 512], BF16, tag="xt")
                    nc.sync.dma_start(
                        xt[:],
                        xT.ap().rearrange("(fo fi) t -> fi fo t", fi=128)[:, :, ts(tch, 512)],
                    )
                    kss_ps = kvq.tile([HKV, 512], FP32, tag="kss")
                    pend_kv = None
                    for m in range(FKV):
                        wm = kvw.tile([128, FO, 128], BF16, tag="wm")
                        nc.sync.dma_start(
                            wm[:],
                            wkv.ap().rearrange("(fo fi) o -> fi fo o", fi=128)[:, :, ts(m, 128)],
                        )
                        ps = kvp.tile([128, 512], FP32, tag="ps")
                        for fo in range(FO):
                            nc.tensor.matmul(
                                ps[:], wm[:, fo, :], xt[:, fo, :],
                                start=(fo == 0), stop=(fo == FO - 1),
                            )
                        if pend_kv is not None:
                            pend_kv()
                            pend_kv = None
                        if m < HKV:
                            sq = kvs.tile([128, 512], BF16, tag="sq")
                            nc.scalar.square(sq[:], ps[:])
                            ca = kvs.tile([128, 512], FP32, tag="ca")
                            nc.vector.tensor_mul(ca[:], ps[:], cosk_sb[:, ts(tch, 512)])
                            cb = kvs.tile([128, 512], FP32, tag="cb")
                            nc.vector.tensor_mul(
                                cb[0:HALF, :], ps[HALF:128, :], sink_sb[0:HALF, ts(tch, 512)]
                            )
                            nc.vector.tensor_mul(
                                cb[HALF:128, :], ps[0:HALF, :], sink_sb[HALF:128, ts(tch, 512)]
                            )
                            nc.vector.tensor_add(kT_sb[:, m, ts(tch, 512)], ca[:], cb[:])

                            def _kfin(sq=sq, m=m):
                                nc.tensor.matmul(
                                    kss_ps[:], oh4_sb[:, ts(m, HKV)], sq[:],
                                    start=(m == 0), stop=(m == HKV - 1),
                                )

                            pend_kv = _kfin
                        else:
                            hh = m - HKV
                            vt = kvs.tile([128, 512], BF16, tag="vt")
                            nc.scalar.copy(vt[:], ps[:])

                            def _vfin(vt=vt, hh=hh, tch=tch):
                                vtp = kvq.tile([128, 512], BF16, tag="vtp")
                                for sub in range(4):
                                    nc.tensor.transpose(
                                        vtp[:, ts(sub, 128)], vt[:, ts(sub, 128)], idb_sb[:]
                                    )
                                nc.vector.tensor_copy(
                                    v_sb[:, hh, ds(tch * 4, 4), :].rearrange(
                                        "p tb d -> p (tb d)"
                                    ),
                                    vtp[:],
                                )

                            pend_kv = _vfin
                    pend_kv()
                    # per-key exp scale: 1/sqrt(kss + D*eps) (= rms-norm * 1/sqrt(D))
                    srt = kvs.tile([HKV, 512], FP32, tag="srt")
                    nc.scalar.activation(srt[:], kss_ps[:], AF.Sqrt, bias=epsd_sb[0:HKV, :])
                    nc.vector.reciprocal(binv_sb[:, ts(tch, 512)], srt[:])
                    for u in range(4):
                        tb = tch * 4 + u
                        btp = kvq.tile([128, HKV], FP32, tag="btp")
                        nc.tensor.transpose(btp[:], binv_sb[:, ts(tb, 128)], idf_sb[0:HKV, 0:HKV])
                        nc.vector.tensor_copy(bT_sb[:, tb, :], btp[:])

            # =============== Phase A: causal attention (transposed softmax) =======
            with (
                tc.tile_pool(name="asb", bufs=3) as asb,
                tc.tile_pool(name="ap1", bufs=3, space="PSUM") as ap1,
                tc.tile_pool(name="ap2", bufs=2, space="PSUM") as ap2,
                tc.tile_pool(name="ap3", bufs=2, space="PSUM") as ap3,
            ):
                for h in range(H):
                    g = h // G
                    o_ps = ap2.tile([128, TSL], FP32, tag="o")
                    s_sum = ap3.tile([1, TSL], FP32, tag="sum")
                    pend_a = None
                    for j in range(TB):
                        i, dg = j // 4, j % 4
                        off = 128 * i + 32 * dg
                        s_ps = ap1.tile([128, TSL], FP32, tag="s")
                        nc.tensor.matmul(
                            s_ps[:, off:TSL],
                            kT_sb[:, g, ts(j, 128)],
                            qT_sb[:, h, ds(off, TSL - off)],
                            start=True, stop=True,
                        )
                        if pend_a is not None:
                            pend_a()
                            pend_a = None
                        pT = asb.tile([128, TSL], BF16, tag="pT")
                        nc.scalar.activation(
                            pT[:, off:TSL], s_ps[:, off:TSL], AF.Exp,
                            scale=bT_sb[:, j, g : g + 1],
                        )
                        nc.gpsimd.tensor_mul(
                            pT[:, ds(off, 128 - 32 * dg)],
                            pT[:, ds(off, 128 - 32 * dg)],
                            mask_sb[:, ds(128 * dg + 32 * dg, 128 - 32 * dg)],
                        )

                        def _av(pT=pT, j=j, off=off, g=g, o_ps=o_ps, s_sum=s_sum):
                            nc.tensor.matmul(
                                o_ps[:, off:TSL], v_sb[:, g, j, :], pT[:, off:TSL],
                                start=(j == 0), stop=(j == TB - 1),
                            )
                            nc.tensor.matmul(
                                s_sum[:, off:TSL], ones_sb[:], pT[:, off:TSL],
                                start=(j == 0), stop=(j == TB - 1),
                            )

                        pend_a = _av
                    pend_a()
                    rs = asb.tile([1, TSL], FP32, tag="rs")
                    nc.vector.reciprocal(rs[:], s_sum[:])
                    rb = asb.tile([128, TSL], FP32, tag="rb")
                    nc.gpsimd.partition_broadcast(rb[:], rs[:])
                    nc.vector.tensor_mul(yT_sb[:, h, :], o_ps[:], rb[:])

            # =============== Phase P: output projection ===========================
            with (
                tc.tile_pool(name="pw", bufs=2) as pw,
                tc.tile_pool(name="po", bufs=3) as po,
                tc.tile_pool(name="pp", bufs=4, space="PSUM") as pp,
            ):
                for oc in range(NOC):
                    wpo = pw.tile([128, H, 512], BF16, tag="wpo")
                    nc.sync.dma_start(
                        wpo[:],
                        wp.ap().rearrange("(fo fi) o -> fi fo o", fi=128)[:, :, ts(oc, 512)],
                    )
                    for tau in range(NI):
                        ops = pp.tile([128, 512], FP32, tag="ops")
                        for h in range(H):
                            nc.tensor.matmul(
                                ops[:], yT_sb[:, h, ts(tau, 128)], wpo[:, h, :],
                                start=(h == 0), stop=(h == H - 1),
                            )
                        osb = po.tile([128, 512], FP32, tag="osb")
                        if tau % 2 == 0:
                            nc.vector.tensor_copy(osb[:], ops[:])
                        else:
                            nc.scalar.copy(osb[:], ops[:])
                        nc.sync.dma_start(out.ap()[ts(tau, 128), ts(oc, 512)], osb[:])

    nc.compile()
    return nc


def _rope_tables_fm(T: int):
    """Feat-major rope tables [128, T]; sin sign baked for the rotate-half form."""
    if T > TRAIN_LEN:
        scale = T / TRAIN_LEN
        base = ROPE_BASE * scale ** (D / (D - 2))
    else:
        base = ROPE_BASE
    inv_freq = 1.0 / base ** (np.arange(0, D, 2, dtype=np.float32) / D)  # [HALF]
    freqs = np.outer(inv_freq, np.arange(T, dtype=np.float32))  # [HALF, T]
    cos = np.cos(freqs).astype(np.float32)
    sin = np.sin(freqs).astype(np.float32)
    cosT = np.concatenate([cos, cos], axis=0)  # [128, T]
    sinT = np.concatenate([sin, -sin], axis=0)
    return cosT, sinT


def make_in_maps(x, Wq, Wk, Wv, Wproj, q_gain, T: int):
    TSL = T // GRP
    cosT, sinT = _rope_tables_fm(T)
    bf = ml_dtypes.bfloat16
    wq_in = np.ascontiguousarray(Wq.T).astype(bf)
    wkv_in = np.ascontiguousarray(np.concatenate([Wk, Wv], axis=0).T).astype(bf)
    wp_in = np.ascontiguousarray(Wproj.T).astype(bf)
    idf = np.eye(128, dtype=np.float32)
    idb = np.eye(128, dtype=bf)
    oh4 = np.zeros((128, 16), dtype=bf)
    for m in range(HKV):
        oh4[:, HKV * m + m] = 1.0
    gsd = (np.asarray(q_gain, dtype=np.float32) * np.sqrt(D)).reshape(1, H)
    xTs = [np.ascontiguousarray(x[b].T).astype(bf) for b in range(x.shape[0])]

    in_maps = []
    for c in range(NCORES):
        b, s = c // GRP, c % GRP
        rows = (
            512 * np.repeat(np.arange(TSL // 128), 128)
            + s
            + 4 * np.tile(np.arange(128), TSL // 128)
        )
        kk = np.arange(128)[:, None]
        r = np.arange(128)[None, :]
        maskd = np.zeros((128, 512), dtype=bf)
        for dg in range(4):
            maskd[:, 128 * dg : 128 * (dg + 1)] = (128 * dg + kk <= s + 4 * r).astype(bf)
        in_maps.append(
            {
                "xT": xTs[b],
                "xTg": np.ascontiguousarray(xTs[b][:, rows]),
                "wq": wq_in,
                "wkv": wkv_in,
                "wp": wp_in,
                "cosq": np.ascontiguousarray(cosT[:, rows]),
                "sinq": np.ascontiguousarray(sinT[:, rows]),
                "cosk": cosT,
                "sink": sinT,
                "maskd": maskd,
                "oh4": oh4,
                "idf": idf,
                "idb": idb,
                "gsd": gsd,
            }
        )
    return in_maps


_NC_CACHE = {}


def run(x, Wq, Wk, Wv, Wproj, q_gain, T=None, use_collective=True, **spmd_kwargs):
    T = T if T is not None else x.shape[1]
    key = (T, True)
    if key not in _NC_CACHE:
        _NC_CACHE[key] = build_nc(T)
    nc = _NC_CACHE[key]
    in_maps = make_in_maps(x, Wq, Wk, Wv, Wproj, q_gain, T)
    res = run_bass_kernel_spmd(nc, in_maps, core_ids=list(range(NCORES)), **spmd_kwargs)
    TSL = T // GRP
    out = np.empty((x.shape[0], T, DIM), dtype=np.float32)
    for c in range(NCORES):
        b, s = c // GRP, c % GRP
        rows = (
            512 * np.repeat(np.arange(TSL // 128), 128)
            + s
            + 4 * np.tile(np.arange(128), TSL // 128)
        )
        out[b, rows, :] = res.results[c]["out"]
    return out, res


def kernel(x, Wq, Wk, Wv, Wproj, q_gain):
    x = np.asarray(x, dtype=np.float32)
    out, _ = run(
        x,
        np.asarray(Wq, dtype=np.float32),
        np.asarray(Wk, dtype=np.float32),
        np.asarray(Wv, dtype=np.float32),
        np.asarray(Wproj, dtype=np.float32),
        np.asarray(q_gain, dtype=np.float32),
    )
    return out
